# revision 16
# baseline (speedup 1.0000x reference)
"""Multi-head attention (B=2, S=2048, D=768, H=12) on 8 NeuronCores.

Sharding: data-parallel over batch (2) x tensor-parallel over heads (4 groups
of 3 heads) = 8 cores. Each core computes its 3 heads' Q/K/V projections,
attention, and a partial output projection; the host sums the 4 per-batch
partials and adds the output bias.

All SBUF operands are fp16 (PE fast mode + FWL; PSUM accumulation stays
fp32). The schedule keeps ScalarE (the Exp pipeline, ~110us floor) busy
end-to-end and hides everything else in the PE's slack behind it:
  - inputs are host-permuted to [partition, chunk, ...] layouts so every
    DMA is a contiguous large-descriptor transfer; weights ride the
    sync-engine HWDGE ring while x rides the scalar-engine ring in two
    pieces, overlapping the first Q/K matmuls
  - K m=0 tiles project before Q, each tile evicting immediately after its
    last accumulation matmul, so head-0 scores chain on with no PE gap
    (keeping the HAM clock warm into attention)
  - the m=1 (head 2) Q/K tiles run as column-tiled pairs (Q in array cols
    0-63, K in 64-127, concurrently) inside the head-1 loop; the V
    projection rides the head-0 loop; the cpair-0 output projection rides
    the cpair-1 head-0 loop
  - attention is split by sq column pairs (2x 1024 cols): scoresT
    [sk 128, 1024] per (cpair, head, sk-chunk) in PSUM -> one Exp on
    ScalarE (scale folded in) -> ctx accumulation [65, 512] with a ones
    column in V giving softmax denominators for free; normalization uses a
    PE rank-1 broadcast of the reciprocal row (no GpSimd in the chain)
  outT [768, 2048] fp16 partial output projection, host-summed across
    head groups in fp32
"""

import sys

sys.path.insert(0, "/opt/trn_rl_repo")

import numpy as np

B, S, D = 2, 2048, 768
H, DK = 12, 64
P = 128
HG = 3              # heads per core
E = HG * DK         # 192: per-core projection width
KD = D // P         # 6 contraction chunks
SQC = S // 512      # 4 sq chunks of 512
SKC = S // P        # 16 sk chunks of 128
SCALE = 1.0 / 8.0   # 1/sqrt(DK)

_NC_CACHE = {}


def _build_bass(body_reps=1):
    import concourse.bacc as bacc
    import concourse.tile as tile
    from concourse import mybir

    f16 = mybir.dt.float16
    f32 = mybir.dt.float32
    Exp = mybir.ActivationFunctionType.Exp

    nc = bacc.Bacc(trn_type="TRN2", debug=False)

    # host-permuted: row p holds [KD, ...] chunk-contiguous data
    xP = nc.dram_tensor("xP", [P, KD * S], f16, kind="ExternalInput")
    wP = nc.dram_tensor("wP", [P, KD * 3 * E], f16, kind="ExternalInput")
    bqkv = nc.dram_tensor("bqkv", [1, 3 * E], f16, kind="ExternalInput")
    woT = nc.dram_tensor("woT", [E, D], f16, kind="ExternalInput")
    ones_d = nc.dram_tensor("ones", [P, 512], f16, kind="ExternalInput")
    outT = nc.dram_tensor("outT", [D, S], f16, kind="ExternalOutput")

    xP_d = xP.ap().rearrange("p (c s) -> p c s", s=S)
    wP_d = wP.ap().rearrange("p (c e) -> p c e", e=3 * E)
    outT_d = outT.ap().rearrange("(c p) s -> p c s", p=P)

    with tile.TileContext(nc) as tc:
        for _rep in range(body_reps):
            with tc.tile_pool(name="persist", bufs=1) as persist, \
                 tc.tile_pool(name="work", bufs=4) as work, \
                 tc.tile_pool(name="small", bufs=2) as small:

                # ---- batched input DMAs on two parallel HWDGE rings ----
                wqkv = persist.tile([P, KD, 3 * E], f16, tag="wqkv")
                nc.sync.dma_start(out=wqkv[:], in_=wP_d)
                bqkv_sb = persist.tile([1, 3 * E], f16, tag="bqkv")
                nc.sync.dma_start(out=bqkv_sb[:], in_=bqkv.ap())
                ones = persist.tile([P, 512], f16, tag="ones")
                nc.sync.dma_start(out=ones[:], in_=ones_d.ap())
                wo_a = persist.tile([P, D], f16, tag="wo_a")
                nc.sync.dma_start(out=wo_a[:], in_=woT.ap()[0:P, :])
                wo_b = persist.tile([64, D], f16, tag="wo_b")
                nc.sync.dma_start(out=wo_b[:], in_=woT.ap()[P:E, :])

                x_all = persist.tile([P, KD, S], f16, tag="x")
                nc.scalar.dma_start(out=x_all[:, 0:3, :], in_=xP_d[:, 0:3, :])
                nc.scalar.dma_start(out=x_all[:, 3:KD, :], in_=xP_d[:, 3:KD, :])

                # preload the Exp table while the x DMAs run
                warm = small.tile([1, 16], f16, tag="warm")
                nc.scalar.activation(warm[:], ones[0:1, 0:16], Exp, scale=1.0)

                # ---- persistent activations ----
                # qt split per column pair so cp0 attention doesn't wait on
                # the cp1 eviction
                qt_a = [persist.tile([P, 1024], f16, tag=f"qt_a{cp}",
                                     name=f"qt_a{cp}") for cp in range(2)]
                qt_b = [persist.tile([64, 1024], f16, tag=f"qt_b{cp}",
                                     name=f"qt_b{cp}") for cp in range(2)]
                kt_a = [persist.tile([P, 1024], f16, tag=f"kt_a{g}",
                                     name=f"kt_a{g}") for g in range(2)]
                kt_b = persist.tile([64, S], f16, tag="kt_b")
                v_sb = [persist.tile([P, HG, 65], f16, tag=f"v{i}", name=f"v{i}")
                        for i in range(SKC)]
                ctx_a = persist.tile([P, S], f16, tag="ctx_a")
                ctx_b = persist.tile([64, S], f16, tag="ctx_b")

                # packed column order: Qm0 | Km0 | Qm1 | Km1 | V
                def w_slice(d, which, m, mw):
                    off = which * P if m == 0 else 2 * P + which * 64
                    return wqkv[:, d, off : off + mw]

                def b_slice(which, m, mw):
                    off = which * P if m == 0 else 2 * P + which * 64
                    return bqkv_sb[0:1, off : off + mw]

                def wm1_slice(d):
                    return wqkv[:, d, 2 * P : 2 * P + P]

                def bm1_slice():
                    return bqkv_sb[0:1, 2 * P : 2 * P + P]

                def qdst(c):
                    return qt_a[c // 2][:, (c % 2) * 512 : (c % 2) * 512 + 512]

                def kdst(c):
                    return kt_a[c // 2][:, (c % 2) * 512 : (c % 2) * 512 + 512]

                # ====== Q/K m=0 projections (heads 0,1), K first ======
                # pass 1: d=0..2 accumulation for all 8 tiles (first x piece)
                # pass 2: per tile d=3..5 + bias + immediate eviction, K tiles
                # first, so attention chains on with no PE gap.
                with tc.tile_pool(name="proj_ps", bufs=8, space="PSUM") as proj_ps:
                    ps = []
                    for which in (1, 0):     # K tiles first, then Q
                        for c in range(SQC):
                            ps.append(proj_ps.tile(
                                [P, 512], f32, tag="proj",
                                name=f"proj_{which}_{c}"))
                    k = 0
                    for which in (1, 0):
                        for c in range(SQC):
                            for d in range(3):
                                nc.tensor.matmul(
                                    ps[k][:],
                                    w_slice(d, which, 0, P),
                                    x_all[:, d, c * 512 : (c + 1) * 512],
                                    start=(d == 0), stop=False,
                                )
                            k += 1
                    for which, c in ((1, 0), (1, 1), (0, 0), (0, 1),
                                     (1, 2), (1, 3), (0, 2), (0, 3)):
                        k = (1 - which) * SQC + c
                        for d in range(3, KD):
                            nc.tensor.matmul(
                                ps[k][:],
                                w_slice(d, which, 0, P),
                                x_all[:, d, c * 512 : (c + 1) * 512],
                                start=False, stop=False,
                            )
                        nc.tensor.matmul(
                            ps[k][:],
                            b_slice(which, 0, P),
                            ones[0:1, 0:512],
                            start=False, stop=True,
                        )
                        dst = kdst(c) if which == 1 else qdst(c)
                        nc.vector.tensor_copy(dst, ps[k][:])

                # ====== attention ======
                def v_proj(i, pool):
                    vps = pool.tile([P, E], f32, tag="ctx", name=f"vps_{i}")
                    for d in range(KD):
                        nc.tensor.matmul(
                            vps[:],
                            x_all[:, d, i * P : (i + 1) * P],
                            wv_col(d),
                            start=(d == 0), stop=False,
                        )
                    nc.tensor.matmul(
                        vps[:], ones[0:1, 0:P], bqkv_sb[0:1, 2 * E : 3 * E],
                        start=False, stop=True,
                    )
                    nc.vector.tensor_copy(
                        v_sb[i][:, :, 64:65], ones[:, 0:3][:, :, None]
                    )
                    nc.vector.tensor_copy(
                        v_sb[i][:, :, 0:64],
                        vps[:, 0:E].rearrange("p (h d) -> p h d", h=HG),
                    )

                def wv_col(d):
                    return wqkv[:, d, 2 * E : 3 * E]   # V at cols 384:576

                def m1_tile(c, pool):
                    # merged Q|K m=1 projection for sq chunk c: output rows
                    # 0-63 = head-2 Q, 64-127 = head-2 K (adjacent packed
                    # weight columns -> one full-width matmul per d chunk)
                    mp = pool.tile([P, 512], f32, tag="ctx", name=f"m1_{c}")
                    for d in range(KD):
                        nc.tensor.matmul(
                            mp[:],
                            wm1_slice(d),
                            x_all[:, d, c * 512 : (c + 1) * 512],
                            start=(d == 0), stop=False,
                        )
                    nc.tensor.matmul(
                        mp[:], bm1_slice(), ones[0:1, 0:512],
                        start=False, stop=True,
                    )
                    nc.vector.tensor_copy(
                        qt_b[c // 2][:, (c % 2) * 512 : (c % 2) * 512 + 512],
                        mp[0:64, :])
                    nc.vector.tensor_copy(
                        kt_b[:, c * 512 : (c + 1) * 512], mp[64:128, :])

                def out_proj_unit(c, ep, pool, use_act):
                    # output projection for sq chunk c, e-pair ep
                    o = work.tile([P, 2, 512], f16, tag="o", bufs=4,
                                  name=f"o_{c}_{ep}")
                    for k in range(2):
                        e = 2 * ep + k
                        op = pool.tile([P, 512], f32, tag="ctx",
                                       name=f"op_{e}_{c}")
                        nc.tensor.matmul(
                            op[:],
                            wo_a[:, e * P : (e + 1) * P],
                            ctx_a[:, c * 512 : (c + 1) * 512],
                            start=True, stop=False,
                        )
                        nc.tensor.matmul(
                            op[:],
                            wo_b[:, e * P : (e + 1) * P],
                            ctx_b[:, c * 512 : (c + 1) * 512],
                            start=False, stop=True,
                        )
                        if use_act and k % 2 == 1:
                            nc.scalar.activation(
                                o[:, k, :], op[:],
                                mybir.ActivationFunctionType.Copy,
                            )
                        else:
                            nc.vector.tensor_copy(o[:, k, :], op[:])
                    nc.sync.dma_start(
                        out=outT_d[:, 2 * ep : 2 * ep + 2,
                                   c * 512 : (c + 1) * 512],
                        in_=o[:],
                    )

                with tc.tile_pool(name="sc_ps", bufs=2, space="PSUM") as sc_ps, \
                     tc.tile_pool(name="ctx_ps", bufs=4, space="PSUM") as ctx_ps:

                    def kt_sl(h, i):
                        if h < 2:
                            return kt_a[i // 8][h * 64 : (h + 1) * 64,
                                               (i % 8) * P : (i % 8) * P + P]
                        return kt_b[0:64, i * P : (i + 1) * P]

                    def normalize(cps_j, ctx_h, c):
                        den = small.tile([1, 512], f32, tag="den")
                        nc.vector.tensor_copy(den[:], cps_j[64:65, :])
                        r = small.tile([1, 512], f32, tag="r")
                        nc.vector.reciprocal_approx_fast(r[:], den[:])
                        rb = small.tile([64, 512], f32, tag="rb")
                        nc.gpsimd.partition_broadcast(rb[:], r[:])
                        nc.vector.tensor_mul(
                            ctx_h[:, c * 512 : (c + 1) * 512],
                            cps_j[0:64, :],
                            rb[:],
                        )

                    et0 = []        # head-0 cp0 exp tiles, consumed in h2 loop
                    for cp in range(2):        # sq column pair
                        for h in range(HG):
                            if h < 2:
                                qt_h = qt_a[cp][h * 64 : (h + 1) * 64, :]
                                ctx_h = ctx_a[h * 64 : (h + 1) * 64, :]
                            else:
                                qt_h = qt_b[cp][0:64, :]
                                ctx_h = ctx_b[0:64, :]
                            defer0 = (cp == 0 and h == 0)

                            if not defer0:
                                cps = [ctx_ps.tile([65, 512], f32, tag="ctx",
                                                   name=f"cps_{cp}_{h}_{j}")
                                       for j in range(2)]
                            if cp == 0 and h == 2:
                                cps0 = [ctx_ps.tile([65, 512], f32, tag="ctx",
                                                    name=f"cps0_{j}")
                                        for j in range(2)]
                            for i in range(SKC):
                                sp = sc_ps.tile([P, 1024], f32, tag="sc",
                                                name=f"sp_{cp}_{h}_{i}")
                                for j in range(2):
                                    nc.tensor.matmul(
                                        sp[:, j * 512 : (j + 1) * 512],
                                        kt_sl(h, i),
                                        qt_h[:, j * 512 : (j + 1) * 512],
                                        start=True, stop=True,
                                    )
                                if defer0:
                                    et = work.tile([P, 1024], f16, tag="exp0",
                                                   bufs=16, name=f"et0_{i}")
                                    et0.append(et)
                                else:
                                    et = work.tile([P, 1024], f16, tag="exp",
                                                   name=f"et_{cp}_{h}_{i}")
                                nc.scalar.activation(et[:], sp[:], Exp, scale=SCALE)
                                if defer0:
                                    v_proj(i, ctx_ps)
                                    continue
                                if cp == 0 and h == 1 and i % 4 == 2:
                                    m1_tile(i // 4, ctx_ps)
                                if cp == 1 and i in (3, 9) and h < 2:
                                    # cp0 output projection: 2 units per head
                                    t = h * 2 + (i == 9)
                                    out_proj_unit(t // 3, t % 3, ctx_ps, False)
                                if cp == 1 and h == 2 and i in (3, 9):
                                    t = 4 + (i == 9)
                                    out_proj_unit(t // 3, t % 3, ctx_ps, False)
                                if cp == 0 and h == 2:
                                    # deferred head-0 ctx accumulation
                                    for j in range(2):
                                        nc.tensor.matmul(
                                            cps0[j][:],
                                            v_sb[i][:, 0, :],
                                            et0[i][:, j * 512 : (j + 1) * 512],
                                            start=(i == 0), stop=(i == SKC - 1),
                                        )
                                for j in range(2):
                                    nc.tensor.matmul(
                                        cps[j][:],
                                        v_sb[i][:, h, :],
                                        et[:, j * 512 : (j + 1) * 512],
                                        start=(i == 0), stop=(i == SKC - 1),
                                    )
                            if defer0:
                                continue
                            if cp == 0 and h == 2:
                                for j in range(2):
                                    normalize(cps0[j][:], ctx_a[0:64, :], j)
                            for j in range(2):
                                c = cp * 2 + j
                                normalize(cps[j][:], ctx_h, c)
                                if cp == 1 and h == HG - 1:
                                    for ep in range(3):
                                        out_proj_unit(c, ep, ctx_ps, True)

    nc.finalize()
    return nc


def _get_nc(body_reps=1):
    key = ("nc", body_reps)
    if key not in _NC_CACHE:
        _NC_CACHE[key] = _build_bass(body_reps)
    return _NC_CACHE[key]


def _core_inputs(c, x, w_q, b_q, w_k, b_k, w_v, b_v, w_o):
    b, g = divmod(c, 4)
    gs = slice(g * E, (g + 1) * E)
    # packed columns [Qm0|Km0|Qm1|Km1|V] -> [P, KD*3E] chunk-major rows
    wqkv = np.concatenate(
        [w_q[gs, :].T[:, 0:P], w_k[gs, :].T[:, 0:P],
         w_q[gs, :].T[:, P:E], w_k[gs, :].T[:, P:E],
         w_v[gs, :].T], axis=1
    ).astype(np.float16)
    wP = np.ascontiguousarray(
        wqkv.reshape(KD, P, 3 * E).transpose(1, 0, 2)).reshape(P, KD * 3 * E)
    xT = x[b].T.astype(np.float16)   # [768, 2048]
    xP = np.ascontiguousarray(
        xT.reshape(KD, P, S).transpose(1, 0, 2)).reshape(P, KD * S)
    bqkv = np.concatenate(
        [b_q[gs][0:P], b_k[gs][0:P], b_q[gs][P:E], b_k[gs][P:E],
         b_v[gs]]).reshape(1, 3 * E)
    return {
        "xP": xP,
        "wP": wP,
        "bqkv": bqkv.astype(np.float16),
        "woT": np.ascontiguousarray(w_o[:, gs].T).astype(np.float16),
        "ones": np.ones((P, 512), np.float16),
    }


def kernel(x, w_q, b_q, w_k, b_k, w_v, b_v, w_o, b_o, _trace=False, _debug=False):
    from concourse.bass_utils import run_bass_kernel_spmd

    x = np.asarray(x, np.float32)
    args = [np.asarray(a, np.float32) for a in
            (w_q, b_q, w_k, b_k, w_v, b_v, w_o)]
    b_o = np.asarray(b_o, np.float32)

    nc = _get_nc()
    in_maps = [_core_inputs(c, x, *args) for c in range(8)]
    res = run_bass_kernel_spmd(nc, in_maps, core_ids=list(range(8)), trace=_trace)

    out = np.zeros((B, S, D), np.float32)
    for c in range(8):
        out[c // 4] += res.results[c]["outT"].astype(np.float32).T
    out += b_o
    if _trace:
        kernel._last_results = res
    return out


# revision 18
# speedup vs baseline: 1.0718x; 1.0718x over previous
"""Multi-head attention (B=2, S=2048, D=768, H=12) on 8 NeuronCores.

Sharding: data-parallel over batch (2) x tensor-parallel over heads (4 groups
of 3 heads) = 8 cores. Each core computes its 3 heads' Q/K/V projections,
attention, and a partial output projection; the host sums the 4 per-batch
partials and adds the output bias.

All SBUF operands are fp16 (PE fast mode + FWL; PSUM accumulation stays
fp32). The schedule keeps ScalarE (the Exp pipeline, ~110us floor) busy
end-to-end and hides everything else in the PE's slack behind it:
  - inputs are host-permuted to [partition, chunk, ...] layouts so every
    DMA is a contiguous large-descriptor transfer; weights ride the
    sync-engine HWDGE ring while x rides the scalar-engine ring in two
    pieces, overlapping the first Q/K matmuls
  - K m=0 tiles project before Q, each tile evicting immediately after its
    last accumulation matmul, so head-0 scores chain on with no PE gap
    (keeping the HAM clock warm into attention)
  - the m=1 (head 2) Q/K tiles run as column-tiled pairs (Q in array cols
    0-63, K in 64-127, concurrently) inside the head-1 loop; the V
    projection rides the head-0 loop; the cpair-0 output projection rides
    the cpair-1 head-0 loop
  - attention is split by sq column pairs (2x 1024 cols): scoresT
    [sk 128, 1024] per (cpair, head, sk-chunk) in PSUM -> one Exp on
    ScalarE (scale folded in) -> ctx accumulation [65, 512] with a ones
    column in V giving softmax denominators for free; normalization uses a
    PE rank-1 broadcast of the reciprocal row (no GpSimd in the chain)
  outT [768, 2048] fp16 partial output projection, host-summed across
    head groups in fp32
"""

import sys

sys.path.insert(0, "/opt/trn_rl_repo")

import numpy as np

B, S, D = 2, 2048, 768
H, DK = 12, 64
P = 128
HG = 3              # heads per core
E = HG * DK         # 192: per-core projection width
KD = D // P         # 6 contraction chunks
SQC = S // 512      # 4 sq chunks of 512
SKC = S // P        # 16 sk chunks of 128
SCALE = 1.0 / 8.0   # 1/sqrt(DK)

_NC_CACHE = {}


def _build_bass(body_reps=1):
    import concourse.bacc as bacc
    import concourse.tile as tile
    from concourse import mybir

    f16 = mybir.dt.float16
    f32 = mybir.dt.float32
    Exp = mybir.ActivationFunctionType.Exp

    nc = bacc.Bacc(trn_type="TRN2", debug=False)

    # host-permuted: row p holds [KD, ...] chunk-contiguous data
    xP = nc.dram_tensor("xP", [P, KD * S], f16, kind="ExternalInput")
    wP = nc.dram_tensor("wP", [P, KD * 3 * E], f16, kind="ExternalInput")
    bqkv = nc.dram_tensor("bqkv", [1, 3 * E], f16, kind="ExternalInput")
    woT = nc.dram_tensor("woT", [E, D], f16, kind="ExternalInput")
    ones_d = nc.dram_tensor("ones", [P, 512], f16, kind="ExternalInput")
    outT = nc.dram_tensor("outT", [D, S], f16, kind="ExternalOutput")

    xP_d = xP.ap().rearrange("p (c s) -> p c s", s=S)
    wP_d = wP.ap().rearrange("p (c e) -> p c e", e=3 * E)
    outT_d = outT.ap().rearrange("(c p) s -> p c s", p=P)

    with tile.TileContext(nc) as tc:
        for _rep in range(body_reps):
            with tc.tile_pool(name="persist", bufs=1) as persist, \
                 tc.tile_pool(name="work", bufs=4) as work, \
                 tc.tile_pool(name="small", bufs=2) as small:

                # ---- batched input DMAs on two parallel HWDGE rings ----
                wqkv = persist.tile([P, KD, 3 * E], f16, tag="wqkv")
                nc.sync.dma_start(out=wqkv[:], in_=wP_d)
                bqkv_sb = persist.tile([1, 3 * E], f16, tag="bqkv")
                nc.sync.dma_start(out=bqkv_sb[:], in_=bqkv.ap())
                ones = persist.tile([P, 512], f16, tag="ones")
                nc.sync.dma_start(out=ones[:], in_=ones_d.ap())
                wo_a = persist.tile([P, D], f16, tag="wo_a")
                nc.sync.dma_start(out=wo_a[:], in_=woT.ap()[0:P, :])
                wo_b = persist.tile([64, D], f16, tag="wo_b")
                nc.sync.dma_start(out=wo_b[:], in_=woT.ap()[P:E, :])

                x_all = persist.tile([P, KD, S], f16, tag="x")
                nc.scalar.dma_start(out=x_all[:, :, 0:1024],
                                    in_=xP_d[:, :, 0:1024])
                nc.scalar.dma_start(out=x_all[:, :, 1024:S],
                                    in_=xP_d[:, :, 1024:S])

                # preload the Exp table while the x DMAs run
                warm = small.tile([1, 16], f16, tag="warm")
                nc.scalar.activation(warm[:], ones[0:1, 0:16], Exp, scale=1.0)

                # ---- persistent activations ----
                # qt split per column pair so cp0 attention doesn't wait on
                # the cp1 eviction
                qt_a = [persist.tile([P, 1024], f16, tag=f"qt_a{cp}",
                                     name=f"qt_a{cp}") for cp in range(2)]
                qt_b = [persist.tile([64, 1024], f16, tag=f"qt_b{cp}",
                                     name=f"qt_b{cp}") for cp in range(2)]
                kt_a = [persist.tile([P, 512], f16, tag=f"kt_a{c}",
                                     name=f"kt_a{c}") for c in range(SQC)]
                kt_b = persist.tile([64, S], f16, tag="kt_b")
                v_sb = [persist.tile([P, HG, 65], f16, tag=f"v{i}", name=f"v{i}")
                        for i in range(SKC)]
                ctx_a = persist.tile([P, S], f16, tag="ctx_a")
                ctx_b = persist.tile([64, S], f16, tag="ctx_b")

                # packed column order: Qm0 | Km0 | Qm1 | Km1 | V
                def w_slice(d, which, m, mw):
                    off = which * P if m == 0 else 2 * P + which * 64
                    return wqkv[:, d, off : off + mw]

                def b_slice(which, m, mw):
                    off = which * P if m == 0 else 2 * P + which * 64
                    return bqkv_sb[0:1, off : off + mw]

                def qdst(c):
                    return qt_a[c // 2][:, (c % 2) * 512 : (c % 2) * 512 + 512]

                # ====== Q/K m=0 projections (heads 0,1), K first ======
                # pass 1: d=0..2 accumulation for all 8 tiles (first x piece)
                # pass 2: per tile d=3..5 + bias + immediate eviction, K tiles
                # first, so attention chains on with no PE gap.
                with tc.tile_pool(name="proj_ps", bufs=8, space="PSUM") as proj_ps:
                    for which, c in ((1, 0), (0, 0), (0, 1), (1, 1),
                                     (1, 2), (1, 3), (0, 2), (0, 3)):
                        pt = proj_ps.tile([P, 512], f32, tag="proj",
                                          name=f"proj_{which}_{c}")
                        for d in range(KD):
                            nc.tensor.matmul(
                                pt[:],
                                w_slice(d, which, 0, P),
                                x_all[:, d, c * 512 : (c + 1) * 512],
                                start=(d == 0), stop=False,
                            )
                        nc.tensor.matmul(
                            pt[:],
                            b_slice(which, 0, P),
                            ones[0:1, 0:512],
                            start=False, stop=True,
                        )
                        dst = kt_a[c][:] if which == 1 else qdst(c)
                        nc.vector.tensor_copy(dst, pt[:])

                # ====== attention ======
                def v_proj(i, pool):
                    vps = pool.tile([P, E], f32, tag="ctx", name=f"vps_{i}")
                    for d in range(KD):
                        nc.tensor.matmul(
                            vps[:],
                            x_all[:, d, i * P : (i + 1) * P],
                            wv_col(d),
                            start=(d == 0), stop=False,
                        )
                    nc.tensor.matmul(
                        vps[:], ones[0:1, 0:P], bqkv_sb[0:1, 2 * E : 3 * E],
                        start=False, stop=True,
                    )
                    nc.vector.tensor_copy(
                        v_sb[i][:, :, 64:65], ones[:, 0:3][:, :, None]
                    )
                    nc.vector.tensor_copy(
                        v_sb[i][:, :, 0:64],
                        vps[:, 0:E].rearrange("p (h d) -> p h d", h=HG),
                    )

                def wv_col(d):
                    return wqkv[:, d, 2 * E : 3 * E]

                def m1_tile(c, pool):
                    # merged Q|K m=1 projection for sq chunk c: output rows
                    # 0-63 = head-2 Q, 64-127 = head-2 K (adjacent packed
                    # weight columns -> one full-width matmul per d chunk)
                    mp = pool.tile([P, 512], f32, tag="ctx", name=f"m1_{c}")
                    for d in range(KD):
                        nc.tensor.matmul(
                            mp[:],
                            wqkv[:, d, 2 * P : 3 * P],
                            x_all[:, d, c * 512 : (c + 1) * 512],
                            start=(d == 0), stop=False,
                        )
                    nc.tensor.matmul(
                        mp[:], bqkv_sb[0:1, 2 * P : 3 * P], ones[0:1, 0:512],
                        start=False, stop=True,
                    )
                    nc.vector.tensor_copy(
                        qt_b[c // 2][:, (c % 2) * 512 : (c % 2) * 512 + 512],
                        mp[0:64, :])
                    nc.vector.tensor_copy(
                        kt_b[:, c * 512 : (c + 1) * 512], mp[64:128, :])

                def out_proj_unit(c, ep, pool, use_act):
                    # output projection for sq chunk c, e-pair ep
                    o = work.tile([P, 2, 512], f16, tag="o", bufs=4,
                                  name=f"o_{c}_{ep}")
                    for k in range(2):
                        e = 2 * ep + k
                        op = pool.tile([P, 512], f32, tag="ctx",
                                       name=f"op_{e}_{c}")
                        nc.tensor.matmul(
                            op[:],
                            wo_a[:, e * P : (e + 1) * P],
                            ctx_a[:, c * 512 : (c + 1) * 512],
                            start=True, stop=False,
                        )
                        nc.tensor.matmul(
                            op[:],
                            wo_b[:, e * P : (e + 1) * P],
                            ctx_b[:, c * 512 : (c + 1) * 512],
                            start=False, stop=True,
                        )
                        if use_act and k % 2 == 1:
                            nc.scalar.activation(
                                o[:, k, :], op[:],
                                mybir.ActivationFunctionType.Copy,
                            )
                        else:
                            nc.vector.tensor_copy(o[:, k, :], op[:])
                    nc.sync.dma_start(
                        out=outT_d[:, 2 * ep : 2 * ep + 2,
                                   c * 512 : (c + 1) * 512],
                        in_=o[:],
                    )

                with tc.tile_pool(name="sc_ps", bufs=2, space="PSUM") as sc_ps, \
                     tc.tile_pool(name="ctx_ps", bufs=4, space="PSUM") as ctx_ps:
                    for cp in range(2):        # sq column pair: cols cp*1024 +: 1024
                        for h in range(HG):
                            if h < 2:
                                qt_h = qt_a[cp][h * 64 : (h + 1) * 64, :]
                                ctx_h = ctx_a[h * 64 : (h + 1) * 64, :]
                            else:
                                qt_h = qt_b[cp][0:64, :]
                                ctx_h = ctx_b[0:64, :]

                            cps = [ctx_ps.tile([65, 512], f32, tag="ctx",
                                               name=f"cps_{cp}_{h}_{j}")
                                   for j in range(2)]
                            for i in range(SKC):
                                sp = sc_ps.tile([P, 1024], f32, tag="sc",
                                                name=f"sp_{cp}_{h}_{i}")
                                if h < 2:
                                    kt_i = kt_a[i // 4][h * 64 : (h + 1) * 64,
                                                        (i % 4) * P : (i % 4) * P + P]
                                else:
                                    kt_i = kt_b[0:64, i * P : (i + 1) * P]
                                for j in range(2):
                                    nc.tensor.matmul(
                                        sp[:, j * 512 : (j + 1) * 512],
                                        kt_i,
                                        qt_h[:, j * 512 : (j + 1) * 512],
                                        start=True, stop=True,
                                    )
                                et = work.tile([P, 1024], f16, tag="exp",
                                               name=f"et_{cp}_{h}_{i}")
                                nc.scalar.activation(et[:], sp[:], Exp, scale=SCALE)
                                if cp == 0 and h == 0:
                                    v_proj(i, ctx_ps)
                                if cp == 0 and h == 1 and i % 4 == 2:
                                    m1_tile(i // 4, ctx_ps)
                                if cp == 1 and h == 0 and i % 2 == 0 \
                                        and 2 <= i < 14:
                                    # cp0 output projection: 6 units
                                    t = i // 2 - 1
                                    out_proj_unit(t // 3, t % 3, ctx_ps, False)
                                for j in range(2):
                                    nc.tensor.matmul(
                                        cps[j][:],
                                        v_sb[i][:, h, :],
                                        et[:, j * 512 : (j + 1) * 512],
                                        start=(i == 0), stop=(i == SKC - 1),
                                    )
                            for j in range(2):
                                c = cp * 2 + j
                                den = small.tile([1, 512], f32, tag="den")
                                nc.vector.tensor_copy(den[:], cps[j][64:65, :])
                                r = small.tile([1, 512], f32, tag="r")
                                nc.vector.reciprocal_approx_fast(r[:], den[:])
                                rb = small.tile([64, 512], f32, tag="rb")
                                nc.gpsimd.partition_broadcast(rb[:], r[:])
                                nc.vector.tensor_mul(
                                    ctx_h[:, c * 512 : (c + 1) * 512],
                                    cps[j][0:64, :],
                                    rb[:],
                                )
                                if cp == 1 and h == HG - 1:
                                    # cp1 output projection tail
                                    for ep in range(3):
                                        out_proj_unit(c, ep, ctx_ps, True)

    nc.finalize()
    return nc


def _get_nc(body_reps=1):
    key = ("nc", body_reps)
    if key not in _NC_CACHE:
        _NC_CACHE[key] = _build_bass(body_reps)
    return _NC_CACHE[key]


def _core_inputs(c, x, w_q, b_q, w_k, b_k, w_v, b_v, w_o):
    b, g = divmod(c, 4)
    gs = slice(g * E, (g + 1) * E)
    # packed columns [Qm0|Km0|Qm1|Km1|V] -> [P, KD*3E] chunk-major rows
    wqkv = np.concatenate(
        [w_q[gs, :].T[:, 0:P], w_k[gs, :].T[:, 0:P],
         w_q[gs, :].T[:, P:E], w_k[gs, :].T[:, P:E],
         w_v[gs, :].T], axis=1
    ).astype(np.float16)
    wP = np.ascontiguousarray(
        wqkv.reshape(KD, P, 3 * E).transpose(1, 0, 2)).reshape(P, KD * 3 * E)
    xT = x[b].T.astype(np.float16)   # [768, 2048]
    xP = np.ascontiguousarray(
        xT.reshape(KD, P, S).transpose(1, 0, 2)).reshape(P, KD * S)
    bqkv = np.concatenate(
        [b_q[gs][0:P], b_k[gs][0:P], b_q[gs][P:E], b_k[gs][P:E],
         b_v[gs]]).reshape(1, 3 * E)
    return {
        "xP": xP,
        "wP": wP,
        "bqkv": bqkv.astype(np.float16),
        "woT": np.ascontiguousarray(w_o[:, gs].T).astype(np.float16),
        "ones": np.ones((P, 512), np.float16),
    }


def kernel(x, w_q, b_q, w_k, b_k, w_v, b_v, w_o, b_o, _trace=False, _debug=False):
    from concourse.bass_utils import run_bass_kernel_spmd

    x = np.asarray(x, np.float32)
    args = [np.asarray(a, np.float32) for a in
            (w_q, b_q, w_k, b_k, w_v, b_v, w_o)]
    b_o = np.asarray(b_o, np.float32)

    nc = _get_nc()
    in_maps = [_core_inputs(c, x, *args) for c in range(8)]
    res = run_bass_kernel_spmd(nc, in_maps, core_ids=list(range(8)), trace=_trace)

    out = np.zeros((B, S, D), np.float32)
    for c in range(8):
        out[c // 4] += res.results[c]["outT"].astype(np.float32).T
    out += b_o
    if _trace:
        kernel._last_results = res
    return out
